# revision 1
# baseline (speedup 1.0000x reference)
"""Trainium2 Bass kernel for multi-head attention with RoPE (B=2, S=2048,
D=2048, H=16), distributed over 8 NeuronCores with head tensor-parallelism
and an AllToAll to switch to token-parallelism for the output projection.

kernel(**inputs) takes the full unsharded inputs (as produced by the
reference setup_inputs) and returns the full [2, 2048, 2048] f32 output.

Layout strategy: x is pre-transposed/cast to bf16 [D, T] on the host (same
spirit as the host-side weight transposes), so QKV matmuls stream straight
from SBUF xT tiles with no on-device staging. V is produced directly in
natural [t, hd] layout by swapping matmul operands. The output projection
is split into per-head halves so head-0's half overlaps the second
AllToAll.
"""
import numpy as np
import ml_dtypes
import bass_rust
from concourse import bass, bacc, tile, mybir
from concourse.bass_utils import run_bass_kernel_spmd

bf16 = ml_dtypes.bfloat16
BF16 = mybir.dt.bfloat16
F32 = mybir.dt.float32
AF = mybir.ActivationFunctionType
OP = mybir.AluOpType

B, S, D, H = 2, 2048, 2048, 16
HD = 128                 # head dim
NCORES = 8
HL = H // NCORES         # heads per core = 2
EL = HL * HD             # local projection width = 256
T = B * S                # 4096 flattened tokens
NG = 4                   # 1024-token groups in QKV phase
TG = T // NG             # 1024
NKT = S // 128           # 16 key tiles per batch
NQC = S // 512           # 4 query chunks per batch
NDT = D // 128           # 16 contraction tiles
TL = T // NCORES         # 512 tokens per core after AllToAll
SCALE = float(1.0 / np.sqrt(128.0))

_CACHE = {}


def _build():
    nc = bacc.Bacc("TRN2", target_bir_lowering=False, num_devices=NCORES)

    x_t = nc.dram_tensor("x_t", [D, T], BF16, kind="ExternalInput")
    wq_t = nc.dram_tensor("wq_t", [128, NDT * EL], BF16, kind="ExternalInput")
    wk_t = nc.dram_tensor("wk_t", [128, NDT * EL], BF16, kind="ExternalInput")
    wv_t = nc.dram_tensor("wv_t", [128, NDT * EL], BF16, kind="ExternalInput")
    wo_t = nc.dram_tensor("wo_t", [128, NDT * D], BF16, kind="ExternalInput")
    cos_t = nc.dram_tensor("cos_t", [HD, S], BF16, kind="ExternalInput")
    sin_m = nc.dram_tensor("sin_m", [HD, S], BF16, kind="ExternalInput")
    mask_t = nc.dram_tensor("mask_t", [128, B * NKT], F32, kind="ExternalInput")
    out = nc.dram_tensor("out", [TL, D], F32, kind="ExternalOutput")

    ones_dram = nc.inline_tensor(np.ones((128, 128), dtype=bf16), name="ones")

    with tile.TileContext(nc) as tc:
        with (
            tc.tile_pool(name="dram", bufs=1, space="DRAM") as dram,
            tc.tile_pool(name="consts", bufs=1) as consts,
            tc.tile_pool(name="keep", bufs=1) as keep,
        ):
            # AllToAll buffers, token-split into two halves per head so the
            # first half can land (and feed the output projection) while the
            # second is still on the wire
            a2a_in = [[dram.tile([NCORES, HD, TL // 2], BF16,
                                 tag=f"a2a_in{h}_{hf}", name=f"a2a_in{h}_{hf}")
                       for hf in range(2)] for h in range(HL)]
            a2a_out = [[dram.tile([NCORES, HD, TL // 2], BF16,
                                  tag=f"a2a_out{h}_{hf}", name=f"a2a_out{h}_{hf}")
                        for hf in range(2)] for h in range(HL)]

            ones_sb = consts.tile([128, 128], BF16, tag="ones", name="ones_sb")
            nc.scalar.dma_start(ones_sb[:], ones_dram[:])
            mask_sb = consts.tile([128, B * NKT], F32, tag="mask", name="mask_sb")
            nc.scalar.dma_start(mask_sb[:], mask_t[:])
            cos_sb = consts.tile([128, S], BF16, tag="cos", name="cos_sb")
            sin_sb = consts.tile([128, S], BF16, tag="sin", name="sin_sb")

            # persistent per-head tensors: qT/kT in [hd, t]; v natural packed
            # per 128-token block as [t=128, (eh, hd)] along the free dim
            qT = [keep.tile([128, T], BF16, tag=f"qT{h}", name=f"qT{h}")
                  for h in range(HL)]
            kT = [keep.tile([128, T], BF16, tag=f"kT{h}", name=f"kT{h}")
                  for h in range(HL)]
            vnat = keep.tile([128, 2 * T], BF16, tag="vnat", name="vnat")

            # ---------- phase A+B: QKV projections + RoPE ----------
            with (
                tc.tile_pool(name="wsb", bufs=1) as wpool,
                tc.tile_pool(name="xt", bufs=32) as xtpool,
                tc.tile_pool(name="rope", bufs=3) as rope,
                tc.tile_pool(name="qkps", bufs=6, space="PSUM") as qkps,
                tc.tile_pool(name="vps", bufs=2, space="PSUM") as vps,
            ):
                wsb = {}
                for nm, wt in (("q", wq_t), ("k", wk_t), ("v", wv_t)):
                    wsb[nm] = wpool.tile([128, NDT * EL], BF16, tag=f"w{nm}",
                                         name=f"w{nm}")

                for g in range(NG):
                    g0 = g * TG
                    xts = []
                    for dti in range(NDT):
                        xtile = xtpool.tile([128, TG], BF16, tag="xt", name="xt")
                        if g == 0:
                            # interleave the weight/table loads with the first
                            # group's xt stream so the first matmul chain can
                            # start after ~0.5 MB instead of ~8 MB of DMA
                            if dti < 4:
                                c0 = dti * (NDT * EL // 4)
                                c1 = (dti + 1) * (NDT * EL // 4)
                                nc.sync.dma_start(wsb["q"][:, c0:c1],
                                                  wq_t[:, c0:c1])
                            elif dti == 4:
                                nc.sync.dma_start(wsb["k"][:], wk_t[:])
                            elif dti == 8:
                                nc.sync.dma_start(cos_sb[:], cos_t[:])
                                nc.sync.dma_start(sin_sb[:], sin_m[:])
                        nc.sync.dma_start(
                            xtile[:], x_t[dti * 128:(dti + 1) * 128, g0:g0 + TG])
                        xts.append(xtile)
                        if g == 0 and dti == NDT - 1:
                            # wv is consumed ~15us later than the QK stream;
                            # keep it out of the startup-critical DMA prefix
                            nc.sync.dma_start(wsb["v"][:], wv_t[:])
                    for half in range(2):
                        t0 = g0 + half * 512
                        pos0 = t0 % S
                        chains = [(nm, eh) for nm in ("q", "k")
                                  for eh in range(HL)]
                        if g == 0:
                            # d-outer emission: all four chains consume each
                            # xt tile as it lands, so the PE runs 4 matmuls
                            # per arriving DMA instead of pacing 1:1 with it
                            pss = {c: qkps.tile([128, 512], F32, tag="qkps",
                                                name="qkps") for c in chains}
                            if half == 0:
                                # warm the PE clock (HAM) through the startup
                                # preamble with throwaway matmuls; the first
                                # real start=True matmul clears this bank
                                for _ in range(80):
                                    nc.tensor.matmul(
                                        pss[("q", 0)][:, 0:128],
                                        ones_sb[:], ones_sb[:],
                                        start=True, stop=True)
                            for dti in range(NDT):
                                for nm, eh in chains:
                                    nc.tensor.matmul(
                                        pss[(nm, eh)][:],
                                        wsb[nm][:, dti * EL + eh * 128:
                                                dti * EL + (eh + 1) * 128],
                                        xts[dti][:, half * 512:(half + 1) * 512],
                                        start=(dti == 0), stop=(dti == NDT - 1))
                        else:
                            pss = {}
                        for nm, eh in chains:
                            if g == 0:
                                ps = pss[(nm, eh)]
                            else:
                                ps = qkps.tile([128, 512], F32, tag="qkps",
                                               name="qkps")
                                for dti in range(NDT):
                                    nc.tensor.matmul(
                                        ps[:],
                                        wsb[nm][:, dti * EL + eh * 128:
                                                dti * EL + (eh + 1) * 128],
                                        xts[dti][:, half * 512:(half + 1) * 512],
                                        start=(dti == 0), stop=(dti == NDT - 1))
                            dst = qT[eh] if nm == "q" else kT[eh]
                            tmp = rope.tile([128, 512], F32, tag="ropetmp",
                                            name="ropetmp")
                            nc.vector.tensor_tensor(
                                tmp[:], ps[:], cos_sb[:, pos0:pos0 + 512],
                                OP.mult)
                            u = rope.tile([128, 512], F32, tag="ropeu",
                                          name="ropeu")
                            nc.vector.tensor_tensor(
                                u[0:64, :], ps[64:128, :],
                                sin_sb[0:64, pos0:pos0 + 512], OP.mult)
                            nc.vector.tensor_tensor(
                                u[64:128, :], ps[0:64, :],
                                sin_sb[64:128, pos0:pos0 + 512], OP.mult)
                            nc.vector.tensor_tensor(
                                dst[:, t0:t0 + 512], tmp[:], u[:], OP.add)
                    for tb in range(TG // 128):
                        t0 = g0 + tb * 128
                        ps = vps.tile([128, EL], F32, tag="vps", name="vps")
                        for dti in range(NDT):
                            nc.tensor.matmul(
                                ps[:],
                                xts[dti][:, tb * 128:(tb + 1) * 128],
                                wsb["v"][:, dti * EL:(dti + 1) * EL],
                                start=(dti == 0), stop=(dti == NDT - 1))
                        nc.vector.tensor_copy(
                            vnat[:, t0 * 2:t0 * 2 + EL], ps[:])

            # ---------- phase C: SDPA per (head, batch, 1024-query block) ----------
            ot_sb = {}
            mm_hold = [None]
            with tc.tile_pool(name="late", bufs=1) as late:
                wo_sb = late.tile([128, NDT * D], BF16, tag="wo", name="wo_sb")
                otpool = late
                with (
                    tc.tile_pool(name="E", bufs=16) as epool,
                    tc.tile_pool(name="Epair", bufs=4) as eppool,
                    tc.tile_pool(name="Equad", bufs=3) as eqpool,
                    tc.tile_pool(name="onorm", bufs=3) as onpool,
                    tc.tile_pool(name="rec", bufs=3) as recpool,
                    tc.tile_pool(name="sps", bufs=2, space="PSUM") as spool,
                    tc.tile_pool(name="ops", bufs=2, space="PSUM") as opool,
                    tc.tile_pool(name="dps", bufs=2, space="PSUM") as dpool,
                ):
                    for h in range(HL):
                        for b in range(B):
                            q0 = b * S
                            for qp in range(2):
                                qb = q0 + qp * 1024
                                ops_ps = [opool.tile([128, 512], F32, tag="ops",
                                                     name="opsum")
                                          for _ in range(2)]
                                dps_ps = [dpool.tile([128, 512], F32, tag="dps",
                                                     name="dpsum")
                                          for _ in range(2)]
                                E = []
                                pairs = []
                                quads = []

                                def attn_step(kt):
                                    e_t = E[kt]
                                    vcol = (b * NKT + kt) * EL + h * 128
                                    for qc2 in range(2):
                                        mm_hold[0] = nc.tensor.matmul(
                                            ops_ps[qc2][:],
                                            vnat[:, vcol:vcol + 128],
                                            e_t[:, qc2 * 512:(qc2 + 1) * 512],
                                            start=(kt == 0), stop=(kt == NKT - 1))
                                    if kt % 2 == 1:
                                        ep = eppool.tile([128, 1024], BF16,
                                                         tag="epair", name="epair")
                                        nc.vector.tensor_tensor(
                                            ep[:], E[kt - 1][:], e_t[:], OP.add)
                                        pairs.append(ep)
                                    if kt % 4 == 3:
                                        eq = eqpool.tile([128, 1024], BF16,
                                                         tag="equad", name="equad")
                                        nc.vector.tensor_tensor(
                                            eq[:], pairs[-2][:], pairs[-1][:],
                                            OP.add)
                                        quads.append(eq)
                                    if kt % 8 == 7:
                                        eo8 = eqpool.tile([128, 1024], BF16,
                                                          tag="eoct", name="eoct")
                                        nc.vector.tensor_tensor(
                                            eo8[:], quads[-2][:], quads[-1][:],
                                            OP.add)
                                        for qc2 in range(2):
                                            mm_hold[0] = nc.tensor.matmul(
                                                dps_ps[qc2][:], ones_sb[:],
                                                eo8[:, qc2 * 512:(qc2 + 1) * 512],
                                                start=(kt == 7),
                                                stop=(kt == NKT - 1))

                                for kt in range(NKT):
                                    sp = spool.tile([128, 1024], F32, tag="sps",
                                                    name="spsum")
                                    for qc2 in range(2):
                                        nc.tensor.matmul(
                                            sp[:, qc2 * 512:(qc2 + 1) * 512],
                                            kT[h][:, q0 + kt * 128:
                                                  q0 + (kt + 1) * 128],
                                            qT[h][:, qb + qc2 * 512:
                                                  qb + (qc2 + 1) * 512],
                                            start=True, stop=True)
                                    e_t = epool.tile([128, 1024], BF16, tag="E",
                                                     name="etile")
                                    mcol = b * NKT + kt
                                    nc.scalar.activation(
                                        e_t[:], sp[:], AF.Exp,
                                        bias=mask_sb[:, mcol:mcol + 1],
                                        scale=SCALE)
                                    E.append(e_t)
                                    # lag the PV/denominator consumption two
                                    # score tiles behind the exp producer so
                                    # the exp latency stays off the PE
                                    # critical path
                                    if kt > 1:
                                        attn_step(kt - 2)
                                attn_step(NKT - 2)
                                attn_step(NKT - 1)

                                for qc2 in range(2):
                                    rec = recpool.tile([128, 512], F32, tag="rec",
                                                       name="rec")
                                    nc.vector.reciprocal_approx_fast(
                                        rec[:], dps_ps[qc2][:])
                                    on = onpool.tile([128, 512], BF16, tag="on",
                                                     name="onorm")
                                    nc.vector.tensor_tensor(
                                        on[:], ops_ps[qc2][:], rec[:], OP.mult)
                                    j = b * NQC + qp * 2 + qc2
                                    for hf in range(2):
                                        nc.sync.dma_start(
                                            a2a_in[h][hf][j, :, :],
                                            on[:, hf * 256:(hf + 1) * 256])
                        for hf in range(2):
                            nc.gpsimd.collective_compute(
                                "AllToAll", OP.bypass,
                                replica_groups=[list(range(NCORES))],
                                ins=[a2a_in[h][hf].opt()],
                                outs=[a2a_out[h][hf].opt()],
                            )
                        # fetch this head's a2a output on the sync ring: NOT
                        # the scalar ring — a DMA issue there blocks ScalarE's
                        # exp stream until the collective lands. Blocking the
                        # sync ring is fine: only the LAST a2a_in write gates
                        # the next collective trigger.
                        for hf in range(2):
                            for src in range(NCORES):
                                dd = src * HL + h
                                otile = otpool.tile([128, TL // 2], BF16,
                                                    tag=f"ot{dd}_{hf}",
                                                    name=f"ot{dd}_{hf}")
                                nc.sync.dma_start(otile[:],
                                                  a2a_out[h][hf][src, :, :])
                                ot_sb[(dd, hf)] = otile
                        if h == 0:
                            # wo is only needed at the output projection; fetch it
                            # during SDPA when HBM is otherwise quiet
                            nc.scalar.dma_start(wo_sb[:], wo_t[:])

                # ---------- phase D: output projection, split per head ----------
                # head-0's half runs while head-1's AllToAll is in flight
                with (
                    tc.tile_pool(name="ysb", bufs=1) as ypool,
                    tc.tile_pool(name="ysum", bufs=4) as ysumpool,
                    tc.tile_pool(name="yps", bufs=2, space="PSUM") as yppool,
                ):
                    # ordering anchor: keep the output-projection chains after
                    # the SDPA tail in the PE stream — the scheduler's cost
                    # model undershoots the AllToAll latency and would
                    # otherwise front-run these and stall the PE
                    anchor = mm_hold[0]
                    for hh in range(HL):
                        if hh == 1:
                            # the wait for the second AllToAll exceeds the
                            # 3.4us HAM window every run; keep the PE array
                            # warm through it with throwaway matmuls (the
                            # first real start=True clears the bank). Anchor
                            # them on the last h0 chain's FINAL matmul so
                            # they cannot interleave into that chain.
                            anchor = last_mm
                            warm_yp = yppool.tile([128, 512], F32, tag="yps",
                                                  name="warmyp")
                            for _ in range(24):
                                wmm = nc.tensor.matmul(
                                    warm_yp[:, 0:128], ones_sb[:], ones_sb[:],
                                    start=True, stop=True)
                                bass_rust.add_dep_helper(
                                    wmm.ins, anchor.ins, sync=False,
                                    reason="keep PE warm across a2a#1 wait")
                                anchor = wmm
                        for tt in range(TL // 128):
                            hf, tc128 = divmod(tt, 2)
                            for eo in range(4):
                                yp = yppool.tile([128, 512], F32, tag="yps",
                                                 name="ypsum")
                                for di in range(NCORES):
                                    d = di * HL + hh
                                    mm = nc.tensor.matmul(
                                        yp[:],
                                        ot_sb[(d, hf)][:, tc128 * 128:
                                                       (tc128 + 1) * 128],
                                        wo_sb[:, d * D + eo * 512:
                                              d * D + (eo + 1) * 512],
                                        start=(di == 0), stop=(di == NCORES - 1))
                                    if di == 0:
                                        bass_rust.add_dep_helper(
                                            mm.ins, anchor.ins, sync=False,
                                            reason="order oproj after prior phase")
                                        anchor = mm
                                    last_mm = mm
                                if hh == 0:
                                    y0 = ypool.tile([128, 512], F32,
                                                    tag=f"y0_{tt}_{eo}",
                                                    name=f"y0_{tt}_{eo}")
                                    nc.vector.tensor_copy(y0[:], yp[:])
                                    ot_sb[(0, tt, eo)] = y0
                                else:
                                    ys = ysumpool.tile([128, 512], F32, tag="ysum",
                                                       name="ysum")
                                    nc.vector.tensor_tensor(
                                        ys[:], yp[:], ot_sb[(0, tt, eo)][:], OP.add)
                                    nc.sync.dma_start(
                                        out[tt * 128:(tt + 1) * 128,
                                            eo * 512:(eo + 1) * 512], ys[:])

    nc.compile()
    return nc


def _prep_in_maps(x, cos, sin, attn_mask, wq, wk, wv, wo):
    x_t = np.ascontiguousarray(
        np.asarray(x, np.float32).reshape(T, D).T.astype(bf16))      # [D, T]
    cosT = np.ascontiguousarray(np.asarray(cos[0], np.float32).T)    # [HD, S]
    sinT = np.asarray(sin[0], np.float32).T
    sin_m = np.ascontiguousarray(
        np.concatenate([-sinT[:64], sinT[64:]], axis=0))             # [HD, S]
    mask_t = np.ascontiguousarray(
        np.asarray(attn_mask, np.float32).reshape(B * NKT, 128).T)   # [128, 32]

    def pack(w_sl):
        # [E_out, D] slice -> [128, NDT * E_out] d-tile-major layout
        e_out = w_sl.shape[0]
        return np.ascontiguousarray(
            w_sl.T.reshape(NDT, 128, e_out).transpose(1, 0, 2)
            .reshape(128, NDT * e_out).astype(bf16))

    wo_t = pack(np.asarray(wo, np.float32))
    in_maps = []
    for i in range(NCORES):
        sl = slice(i * EL, (i + 1) * EL)
        in_maps.append({
            "x_t": x_t,
            "wq_t": pack(np.asarray(wq, np.float32)[sl]),
            "wk_t": pack(np.asarray(wk, np.float32)[sl]),
            "wv_t": pack(np.asarray(wv, np.float32)[sl]),
            "wo_t": wo_t,
            "cos_t": cosT.astype(bf16),
            "sin_m": sin_m.astype(bf16),
            "mask_t": mask_t,
        })
    return in_maps


def kernel(x, cos, sin, attn_mask, wq, wk, wv, wo, _trace=False):
    if "nc" not in _CACHE:
        _CACHE["nc"] = _build()
    nc = _CACHE["nc"]
    in_maps = _prep_in_maps(x, cos, sin, attn_mask, wq, wk, wv, wo)
    res = run_bass_kernel_spmd(nc, in_maps, core_ids=list(range(NCORES)),
                               trace=_trace)
    _CACHE["last_result"] = res
    y = np.concatenate([np.asarray(res.results[i]["out"], np.float32)
                        for i in range(NCORES)], axis=0)
    return y.reshape(B, S, D)

